# revision 1
# baseline (speedup 1.0000x reference)
"""CBOW negative-sampling loss on 8 Trainium2 NeuronCores.

Strategy (from sharding hint): replicate the embedding tables, data-parallel
over the batch dim. Each core handles 2048 of the 16384 batch rows.

Host side: u_emb and w_emb are concatenated into one [2V, D] bf16 table so
each group needs a single indirect-DMA gather (w-indices offset by +V); bf16
halves both the HBM gather traffic and the DVE element costs.

Per-core kernel layout:
  - batch row b -> chunk c = b // 128, partition p = b % 128.
  - 16 chunks in groups sized (2,4,5,4,1): ramped so the first gather lands
    early and the last group leaves only a short compute tail. Per group ONE
    indirect gather pulls, per partition, n_chunks x (8 u-rows + 6 w-rows) x
    128 bf16.
  - h = sum of the 8 context embeddings: contiguous binary add-tree over all
    chunks of the group at once (3 DVE instructions).
  - dots: one broadcast-mult [P,n,6,128] (bf16) + one X-reduce -> f32 scores.
  - per group: sign pattern [+1,-1,...] then Exp(-x), Ln(x+1) with accum_out
    -> column g of lp [128, n_groups]: sum of softplus(-x) terms.
  - finale: reduce lp rows, PE ones-matmul to collapse partitions -> [1,1]
    (single-descriptor output DMA), host sums the 8 per-core scalars.

loss = sum_b softplus(-score_b) + sum_{b,k} softplus(+neg_score_bk)
"""

import sys

import numpy as np

sys.path.insert(0, "/opt/trn_rl_repo")

import ml_dtypes  # noqa: E402

from concourse import bacc, bass, mybir, tile  # noqa: E402
from concourse.bass_utils import run_bass_kernel_spmd  # noqa: E402

V, D = 100000, 128
B, C, K = 16384, 8, 5
N_CORES = 8
P = 128
B_LOC = B // N_CORES            # 2048 batch rows per core
N_CHUNK = B_LOC // P            # 16 chunks of 128 rows
GROUPS = (1, 2, 4, 5, 4)        # chunks per indirect-DMA gather group
assert sum(GROUPS) == N_CHUNK
J = 1 + K                       # 6 w-rows per batch row (pos + negs)
R = C + J                       # 14 gathered rows per batch row

_NC_CACHE = {}


def _build_bass():
    nc = bacc.Bacc(
        "TRN2",
        target_bir_lowering=False,
        debug=False,
        dynamic_dma_scratch_size=65536,
    )

    bf16 = mybir.dt.bfloat16
    fp32 = mybir.dt.float32
    X = mybir.AxisListType.X
    ADD = mybir.AluOpType.add
    NG = len(GROUPS)

    emb = nc.dram_tensor("emb_cat", [2 * V, D], bf16, kind="ExternalInput")
    gidx = nc.dram_tensor(
        "gidx", [P, N_CHUNK * R], mybir.dt.int32, kind="ExternalInput"
    )
    loss = nc.dram_tensor("loss_part", [1, 1], fp32, kind="ExternalOutput")

    with tile.TileContext(nc) as tc:
        with (
            tc.tile_pool(name="idx", bufs=1) as idx_pool,
            tc.tile_pool(name="gb", bufs=5) as gb_pool,
            tc.tile_pool(name="m", bufs=3) as m_pool,
            tc.tile_pool(name="sc", bufs=2) as sc_pool,
            tc.tile_pool(name="fin", bufs=1) as fin_pool,
            tc.tile_pool(name="ps", bufs=1, space="PSUM") as ps_pool,
        ):
            ones = fin_pool.tile([P, 1], fp32, tag="ones")
            nc.gpsimd.memset(ones[:], 1.0)
            # exp(-x) for all score cols, filled per group; single Ln at end
            ex_all = fin_pool.tile([P, N_CHUNK * J], fp32, tag="ex_all")

            starts = [sum(GROUPS[:g]) for g in range(NG)]
            gb_t = {}

            ix_all = idx_pool.tile([P, N_CHUNK * R], mybir.dt.int32, tag="ix")
            nc.sync.dma_start(out=ix_all[:], in_=gidx[:])

            def issue_gather(g):
                n = GROUPS[g]
                c0 = starts[g]
                gb = gb_pool.tile([P, n * R * D], bf16, tag="gb")
                nc.gpsimd.indirect_dma_start(
                    out=gb[:],
                    out_offset=None,
                    in_=emb[:],
                    in_offset=bass.IndirectOffsetOnAxis(
                        ap=ix_all[:, c0 * R : (c0 + n) * R], axis=0
                    ),
                )
                gb_t[g] = gb

            issue_gather(0)
            for g in range(NG):
                if g + 1 < NG:
                    issue_gather(g + 1)
                n = GROUPS[g]
                gb = gb_t.pop(g)
                g3 = gb[:].rearrange("p (c e) -> p c e", c=n)  # e = R*D

                # h = sum of the 8 context embeddings (cols 0 : 8D of each
                # chunk block); contiguous binary add-tree, all chunks at
                # once, all on DVE (GpSimd has no bf16 speedup and stalls
                # the chain).
                nc.vector.tensor_add(
                    out=g3[:, :, 0 : 4 * D],
                    in0=g3[:, :, 0 : 4 * D],
                    in1=g3[:, :, 4 * D : 8 * D],
                )
                nc.vector.tensor_add(
                    out=g3[:, :, 0 : 2 * D],
                    in0=g3[:, :, 0 : 2 * D],
                    in1=g3[:, :, 2 * D : 4 * D],
                )
                nc.vector.tensor_add(
                    out=g3[:, :, 0:D],
                    in0=g3[:, :, 0:D],
                    in1=g3[:, :, D : 2 * D],
                )
                h4 = g3[:, :, 0:D]  # [P, n, D]

                # m[p, c, j, d] = w[p, c, j, d] * h[p, c, d]
                w4 = g3[:, :, C * D : R * D].rearrange("p c (j d) -> p c j d", j=J)
                m = m_pool.tile([P, n * J * D], bf16, tag="m")
                m4 = m[:].rearrange("p (c j d) -> p c j d", c=n, j=J)
                nc.vector.tensor_mul(
                    out=m4,
                    in0=w4,
                    in1=h4[:, :, None, :].broadcast_to([P, n, J, D]),
                )
                # pre-fold the innermost 128 -> 16 with bf16 adds (~0.3ns/elem)
                # before the TensorReduce (~1.1ns/elem)
                for w_ in (64, 32, 16):
                    nc.vector.tensor_add(
                        out=m4[:, :, :, 0:w_],
                        in0=m4[:, :, :, 0:w_],
                        in1=m4[:, :, :, w_ : 2 * w_],
                    )
                # raw dots (f32): x = [+pos, -negs], neg sign via the reduce
                sc = sc_pool.tile([P, n * J], fp32, tag="sc")
                sc3 = sc[:].rearrange("p (c j) -> p c j", j=J)
                nc.vector.tensor_reduce(
                    out=sc3[:, :, 0:1], in_=m4[:, :, 0:1, 0:16], axis=X, op=ADD
                )
                nc.vector.tensor_reduce(
                    out=sc3[:, :, 1:J],
                    in_=m4[:, :, 1:J, 0:16],
                    axis=X,
                    op=ADD,
                    negate=True,
                )
                # softplus(-x) = ln(1 + exp(-x)); Exp batched per group (one
                # ACT table), Ln once at end.
                c0 = starts[g]
                nc.scalar.activation(
                    out=ex_all[:, c0 * J : (c0 + n) * J],
                    in_=sc[:],
                    func=mybir.ActivationFunctionType.Exp,
                    scale=-1.0,
                )

            # ln(1 + ex) summed over all 96 cols -> per-partition loss [P,1]
            sp = fin_pool.tile([P, N_CHUNK * J], fp32, tag="sp")
            lp1 = fin_pool.tile([P, 1], fp32, tag="lp1")
            nc.scalar.activation(
                out=sp[:],
                in_=ex_all[:],
                func=mybir.ActivationFunctionType.Ln,
                bias=1.0,
                accum_out=lp1[:],
            )
            # collapse partitions via ones-matmul -> [1,1]
            acc = ps_pool.tile([1, 1], fp32, space="PSUM")
            nc.tensor.matmul(out=acc[:], lhsT=ones[:], rhs=lp1[:], start=True, stop=True)
            out_sb = fin_pool.tile([1, 1], fp32, tag="out")
            nc.vector.tensor_copy(out=out_sb[:], in_=acc[:])
            nc.sync.dma_start(out=loss[:], in_=out_sb[:])

    nc.compile()
    return nc


def _get_nc():
    if "nc" not in _NC_CACHE:
        _NC_CACHE["nc"] = _build_bass()
    return _NC_CACHE["nc"]


def _make_in_maps(pos_u, pos_w, neg_w, u_emb, w_emb):
    pos_u = np.asarray(pos_u).astype(np.int32)
    pos_w = np.asarray(pos_w).astype(np.int32)
    neg_w = np.asarray(neg_w).astype(np.int32)
    u_emb = np.asarray(u_emb, dtype=np.float32)
    w_emb = np.asarray(w_emb, dtype=np.float32)

    emb_cat = np.ascontiguousarray(
        np.concatenate([u_emb, w_emb], axis=0).astype(ml_dtypes.bfloat16)
    )

    in_maps = []
    for i in range(N_CORES):
        sl = slice(i * B_LOC, (i + 1) * B_LOC)
        # per batch row: [8 ctx u-idx | pos_w + V | neg_w + V]  -> R = 14
        rows = np.concatenate(
            [pos_u[sl], pos_w[sl, None] + V, neg_w[sl] + V], axis=1
        )  # [B_LOC, 14]
        # batch row b -> (chunk c = b // 128, partition p = b % 128)
        gidx = rows.reshape(N_CHUNK, P, R).transpose(1, 0, 2).reshape(P, N_CHUNK * R)
        in_maps.append(
            {
                "emb_cat": emb_cat,
                "gidx": np.ascontiguousarray(gidx),
            }
        )
    return in_maps


def _install_axon_profile_shim():
    """Provide antenv.axon_hooks (missing in this image) so trace=True can
    capture NTFF profiles via the axon PJRT .so, and keep trace artifacts
    local instead of uploading to a bucket."""
    import contextlib
    import ctypes
    import types

    import concourse.bass_utils as bu

    bu.upload_artifacts = lambda tmpdir: tmpdir

    try:
        from antenv.axon_hooks import get_axon_ntff_profile_hook  # noqa: F401

        return
    except ImportError:
        pass

    mod = types.ModuleType("antenv.axon_hooks")
    holder = {}
    mod.set_axon_ntff_profile_hook = lambda h: holder.__setitem__("h", h)
    mod.get_axon_ntff_profile_hook = lambda: holder.get("h")
    sys.modules["antenv.axon_hooks"] = mod
    import antenv

    antenv.axon_hooks = mod

    so_path = "/opt/axon/libaxon_pjrt.so"
    lib = ctypes.CDLL(so_path)
    if not hasattr(lib, "axon_start_nrt_profile"):
        return
    lib.axon_start_nrt_profile.argtypes = [
        ctypes.POINTER(ctypes.c_int64),
        ctypes.c_size_t,
    ]
    lib.axon_start_nrt_profile.restype = ctypes.c_int64
    lib.axon_stop_nrt_profile.argtypes = [ctypes.c_char_p]
    lib.axon_stop_nrt_profile.restype = ctypes.c_int64

    @contextlib.contextmanager
    def _hook(output_dir, device_ids):
        import jax

        jax.devices()
        if device_ids:
            ids = (ctypes.c_int64 * len(device_ids))(*device_ids)
            rc = lib.axon_start_nrt_profile(ids, len(device_ids))
        else:
            rc = lib.axon_start_nrt_profile(None, 0)
        if rc != 0:
            raise RuntimeError(f"axon_start_nrt_profile rc={rc}")
        try:
            yield
        finally:
            n = lib.axon_stop_nrt_profile(str(output_dir).encode())
            print(f"profile: {n} file(s) written to {output_dir}")

    mod.set_axon_ntff_profile_hook(_hook)


def _run(in_maps, trace=False):
    if trace:
        _install_axon_profile_shim()
    nc = _get_nc()
    return run_bass_kernel_spmd(nc, in_maps, list(range(N_CORES)), trace=trace)


def kernel(pos_u, pos_w, neg_w, u_emb, w_emb):
    in_maps = _make_in_maps(pos_u, pos_w, neg_w, u_emb, w_emb)
    bkr = _run(in_maps, trace=False)
    total = 0.0
    for r in bkr.results:
        total += float(r["loss_part"].astype(np.float64).sum())
    return np.float32(total)


def kernel_traced(pos_u, pos_w, neg_w, u_emb, w_emb):
    """Like kernel() but returns (loss, BassKernelResults) with HW profile."""
    in_maps = _make_in_maps(pos_u, pos_w, neg_w, u_emb, w_emb)
    bkr = _run(in_maps, trace=True)
    total = 0.0
    for r in bkr.results:
        total += float(r["loss_part"].astype(np.float64).sum())
    return np.float32(total), bkr



# revision 6
# speedup vs baseline: 1.0768x; 1.0768x over previous
"""CBOW negative-sampling loss on 8 Trainium2 NeuronCores.

Strategy: replicate the embedding tables, data-parallel over the batch dim
(2048 of 16384 rows per core).

v2 design (from the v1 trace: DVE was the critical path at ~28us busy, plus
a 1.3us mid-stream Ln table load and an 8us serial ramp):
  - u-table stored fp8e4 (values pre-scaled x64 so they sit in e4m3's normal
    range); w-table bf16. Gather traffic 5.2MB/core vs 7.3MB in v1.
  - The 8-way context sum h moves off DVE onto the idle TensorEngine: 8
    accumulating identity-matmuls per chunk group sum the gathered fp8 u-rows
    into fp32 PSUM exactly; ACT copies PSUM->SBUF bf16 with scale 1/64.
  - DVE keeps only: m = w * h (bf16 2x mode), 3 contiguous folds, one
    fp32 TensorReduce -> raw scores.
  - one fp32 TensorReduce per group (no negate split); ACT computes
    exp(-pos) and exp(+neg) per group into ex_all, one final Ln(1+x) with
    accum_out. Both act tables (Exp, Ln) are warmed by dummy activations at
    t=0 so the 1.3us table loads overlap the preamble instead of the tail.
  - 5 gather groups of (2,4,4,4,2) chunks; per group one u-gather and one
    w-gather (10 Pool DMA_INDIRECT instructions, ~1.1us fixed cost each).
    The last group's mult/fold/reduce runs on GpSimd to shorten the DVE
    tail.
  - PE warmup matmuls keep the PE array out of its low p-state before the
    first real accumulation.

loss = sum_b softplus(-score_b) + sum_{b,k} softplus(+neg_score_bk)
"""

import sys

import numpy as np

sys.path.insert(0, "/opt/trn_rl_repo")

import ml_dtypes  # noqa: E402

from concourse import bacc, bass, mybir, tile  # noqa: E402
from concourse.bass_utils import run_bass_kernel_spmd  # noqa: E402

V, D = 100000, 128
B, C, K = 16384, 8, 5
N_CORES = 8
P = 128
B_LOC = B // N_CORES            # 2048 batch rows per core
N_CHUNK = B_LOC // P            # 16 chunks of 128 rows
GROUPS = (2, 4, 4, 4, 2)        # chunks per gather group (each <=4 for PSUM)
assert sum(GROUPS) == N_CHUNK
J = 1 + K                       # 6 w-rows per batch row (pos + negs)
U_SCALE = 64.0                  # host pre-scale for the fp8 u-table
PAD = 128                       # zero pad rows so degenerate contiguous
                                # window reads past row V-1 stay in-tensor

_NC_CACHE = {}


def _build_bass(debug_dump=False):
    nc = bacc.Bacc(
        "TRN2",
        target_bir_lowering=False,
        debug=False,
        dynamic_dma_scratch_size=65536,
    )

    bf16 = mybir.dt.bfloat16
    fp8 = mybir.dt.float8e4
    fp32 = mybir.dt.float32
    X = mybir.AxisListType.X
    ADD = mybir.AluOpType.add
    EXPF = mybir.ActivationFunctionType.Exp
    LNF = mybir.ActivationFunctionType.Ln
    CP = mybir.ActivationFunctionType.Copy
    NG = len(GROUPS)
    starts = [sum(GROUPS[:g]) for g in range(NG)]

    emb_u = nc.dram_tensor("emb_u", [V + PAD, D], fp8, kind="ExternalInput")
    emb_w = nc.dram_tensor("emb_w", [V + PAD, D], bf16, kind="ExternalInput")
    uix_d = nc.dram_tensor("uix", [P, N_CHUNK * C], mybir.dt.int32, kind="ExternalInput")
    wix_d = nc.dram_tensor("wix", [P, N_CHUNK * J], mybir.dt.int32, kind="ExternalInput")
    ident_d = nc.dram_tensor("ident", [P, P], fp8, kind="ExternalInput")
    loss = nc.dram_tensor("loss_part", [1, 1], fp32, kind="ExternalOutput")
    if debug_dump:
        dbg_h = nc.dram_tensor("dbg_h", [P, N_CHUNK * D], bf16, kind="ExternalOutput")
        dbg_sc = nc.dram_tensor("dbg_sc", [P, N_CHUNK * J], fp32, kind="ExternalOutput")
        dbg_ug = nc.dram_tensor("dbg_ug", [P, N_CHUNK * C * D], fp8, kind="ExternalOutput")
        dbg_wg = nc.dram_tensor("dbg_wg", [P, N_CHUNK * J * D], bf16, kind="ExternalOutput")

    with tile.TileContext(nc) as tc:
        with (
            tc.tile_pool(name="idx", bufs=1) as idx_pool,
            tc.tile_pool(name="gb", bufs=1) as gb_pool,
            tc.tile_pool(name="m", bufs=3) as m_pool,
            tc.tile_pool(name="fin", bufs=1) as fin_pool,
            tc.tile_pool(name="ps", bufs=3, space="PSUM") as ps_pool,
            tc.tile_pool(name="ps1", bufs=1, space="PSUM") as ps1_pool,
        ):
            ones = fin_pool.tile([P, 1], fp32, tag="ones")
            nc.gpsimd.memset(ones[:], 1.0)

            # --- warm both act tables (Exp, Ln) during the preamble ---
            warm_sp = fin_pool.tile([P, 1], fp32, tag="warm_sp")
            nc.scalar.activation(out=warm_sp[:], in_=ones[:], func=EXPF)
            nc.scalar.activation(out=warm_sp[:], in_=ones[:], func=LNF, bias=1.0)

            # --- index + identity loads (SP hw-dge queue) ---
            uix = idx_pool.tile([P, N_CHUNK * C], mybir.dt.int32, tag="uix")
            nc.sync.dma_start(out=uix[:], in_=uix_d[:])
            ident = fin_pool.tile([P, P], fp8, tag="ident")
            nc.sync.dma_start(out=ident[:], in_=ident_d[:])
            wix = idx_pool.tile([P, N_CHUNK * J], mybir.dt.int32, tag="wix")
            nc.sync.dma_start(out=wix[:], in_=wix_d[:])

            # --- PE warmup: keep the array out of its low p-state ---
            warm_ps = ps1_pool.tile([P, P], fp32, space="PSUM")
            for _ in range(12):
                nc.tensor.matmul(
                    out=warm_ps[:], lhsT=ident[:], rhs=ident[:],
                    start=True, stop=True,
                )

            # --- gathered data (whole-kernel tiles; slices per group) ---
            ug_all = gb_pool.tile([P, N_CHUNK * C * D], fp8, tag="ug")
            wg_all = gb_pool.tile([P, N_CHUNK * J * D], bf16, tag="wg")
            h_all = gb_pool.tile([P, N_CHUNK * D], bf16, tag="h")
            sc_all = fin_pool.tile([P, N_CHUNK * J], fp32, tag="sc")
            # exp(-pos) in cols [0:16), exp(+neg) in cols [16:96)
            ex_all = fin_pool.tile([P, N_CHUNK * J], fp32, tag="ex_all")

            # --- per-group gather issue (Pool), U before W ---
            for g in range(NG):
                n, c0 = GROUPS[g], starts[g]
                nc.gpsimd.indirect_dma_start(
                    out=ug_all[:, c0 * C * D : (c0 + n) * C * D],
                    out_offset=None,
                    in_=emb_u[:],
                    in_offset=bass.IndirectOffsetOnAxis(
                        ap=uix[:, c0 * C : (c0 + n) * C], axis=0
                    ),
                )
                nc.gpsimd.indirect_dma_start(
                    out=wg_all[:, c0 * J * D : (c0 + n) * J * D],
                    out_offset=None,
                    in_=emb_w[:],
                    in_offset=bass.IndirectOffsetOnAxis(
                        ap=wix[:, c0 * J : (c0 + n) * J], axis=0
                    ),
                )

            # --- per-group compute ---
            for g in range(NG):
                n, c0 = GROUPS[g], starts[g]
                # h = sum of the 8 context u-rows, on PE via accumulating
                # identity matmuls: psum[b, (c,d)] += ug[b, (c,j,d)] per j.
                ug3 = ug_all[:, c0 * C * D : (c0 + n) * C * D].rearrange(
                    "p (c j d) -> p c j d", c=n, j=C
                )
                hps = ps_pool.tile([P, 4 * D], fp32, space="PSUM", tag="hps")
                for j in range(C):
                    nc.tensor.matmul(
                        out=hps[:, 0 : n * D],
                        lhsT=ident[:],
                        rhs=ug3[:, :, j, :],
                        start=(j == 0),
                        stop=(j == C - 1),
                    )
                # ACT: PSUM fp32 -> SBUF bf16, undo the x64 host scale
                h_sb = h_all[:, c0 * D : (c0 + n) * D]
                nc.scalar.activation(
                    out=h_sb, in_=hps[:, 0 : n * D], func=CP, scale=1.0 / U_SCALE
                )

                # engine for this group's elementwise chain
                eng = nc.gpsimd if g == NG - 1 else nc.vector

                w4 = wg_all[:, c0 * J * D : (c0 + n) * J * D].rearrange(
                    "p (c j d) -> p c j d", c=n, j=J
                )
                h4 = h_sb.rearrange("p (c d) -> p c d", c=n)
                m = m_pool.tile([P, 4 * J * D], bf16, tag="m")
                m4 = m[:, 0 : n * J * D].rearrange("p (c j d) -> p c j d", c=n, j=J)
                eng.tensor_mul(
                    out=m4,
                    in0=w4,
                    in1=h4[:, :, None, :].broadcast_to([P, n, J, D]),
                )
                for w_ in (64, 32, 16):
                    eng.tensor_add(
                        out=m4[:, :, :, 0:w_],
                        in0=m4[:, :, :, 0:w_],
                        in1=m4[:, :, :, w_ : 2 * w_],
                    )
                sc3 = sc_all[:, c0 * J : (c0 + n) * J].rearrange(
                    "p (c j) -> p c j", j=J
                )
                # GpSimd has no free-axis reduce; the last group's reduce
                # runs on DVE too (it's ~100ns).
                nc.vector.tensor_reduce(
                    out=sc3, in_=m4[:, :, :, 0:16], axis=X, op=ADD
                )
                # exp(-pos) and exp(+neg) into disjoint ex_all columns
                nc.scalar.activation(
                    out=ex_all[:, c0 : c0 + n],
                    in_=sc3[:, :, 0:1],
                    func=EXPF,
                    scale=-1.0,
                )
                nc.scalar.activation(
                    out=ex_all[:, N_CHUNK + c0 * K : N_CHUNK + (c0 + n) * K],
                    in_=sc3[:, :, 1:J],
                    func=EXPF,
                    scale=1.0,
                )

            # --- finale: ln(1+ex) summed per partition, then collapse ---
            sp = fin_pool.tile([P, N_CHUNK * J], fp32, tag="sp")
            lp1 = fin_pool.tile([P, 1], fp32, tag="lp1")
            nc.scalar.activation(
                out=sp[:], in_=ex_all[:], func=LNF, bias=1.0, accum_out=lp1[:]
            )
            acc = ps1_pool.tile([1, 1], fp32, space="PSUM")
            nc.tensor.matmul(out=acc[:], lhsT=ones[:], rhs=lp1[:], start=True, stop=True)
            out_sb = fin_pool.tile([1, 1], fp32, tag="out")
            nc.vector.tensor_copy(out=out_sb[:], in_=acc[:])
            nc.sync.dma_start(out=loss[:], in_=out_sb[:])
            if debug_dump:
                nc.sync.dma_start(out=dbg_h[:], in_=h_all[:])
                nc.sync.dma_start(out=dbg_sc[:], in_=sc_all[:])
                nc.sync.dma_start(out=dbg_ug[:], in_=ug_all[:])
                nc.sync.dma_start(out=dbg_wg[:], in_=wg_all[:])

    nc.compile()
    return nc


def _get_nc():
    if "nc" not in _NC_CACHE:
        _NC_CACHE["nc"] = _build_bass()
    return _NC_CACHE["nc"]


def _make_in_maps(pos_u, pos_w, neg_w, u_emb, w_emb):
    pos_u = np.asarray(pos_u).astype(np.int32)
    pos_w = np.asarray(pos_w).astype(np.int32)
    neg_w = np.asarray(neg_w).astype(np.int32)
    u_emb = np.asarray(u_emb, dtype=np.float32)
    w_emb = np.asarray(w_emb, dtype=np.float32)

    emb_u = np.zeros((V + PAD, D), dtype=ml_dtypes.float8_e4m3)
    emb_u[:V] = (u_emb * U_SCALE).astype(ml_dtypes.float8_e4m3)
    emb_w = np.zeros((V + PAD, D), dtype=ml_dtypes.bfloat16)
    emb_w[:V] = w_emb.astype(ml_dtypes.bfloat16)
    ident = np.eye(P, dtype=ml_dtypes.float8_e4m3)

    in_maps = []
    for i in range(N_CORES):
        sl = slice(i * B_LOC, (i + 1) * B_LOC)
        # batch row b -> (chunk c = b // 128, partition p = b % 128)
        uix = (
            pos_u[sl]
            .reshape(N_CHUNK, P, C)
            .transpose(1, 0, 2)
            .reshape(P, N_CHUNK * C)
        )
        wrows = np.concatenate([pos_w[sl, None], neg_w[sl]], axis=1)  # [B_LOC, 6]
        wix = (
            wrows.reshape(N_CHUNK, P, J).transpose(1, 0, 2).reshape(P, N_CHUNK * J)
        )
        in_maps.append(
            {
                "emb_u": emb_u,
                "emb_w": emb_w,
                "uix": np.ascontiguousarray(uix),
                "wix": np.ascontiguousarray(wix),
                "ident": ident,
            }
        )
    return in_maps


def _install_axon_profile_shim():
    """Provide antenv.axon_hooks (missing in this image) so trace=True can
    capture NTFF profiles via the axon PJRT .so, and keep trace artifacts
    local instead of uploading to a bucket."""
    import contextlib
    import ctypes
    import types

    import concourse.bass_utils as bu

    bu.upload_artifacts = lambda tmpdir: tmpdir

    try:
        from antenv.axon_hooks import get_axon_ntff_profile_hook  # noqa: F401

        return
    except ImportError:
        pass

    mod = types.ModuleType("antenv.axon_hooks")
    holder = {}
    mod.set_axon_ntff_profile_hook = lambda h: holder.__setitem__("h", h)
    mod.get_axon_ntff_profile_hook = lambda: holder.get("h")
    sys.modules["antenv.axon_hooks"] = mod
    import antenv

    antenv.axon_hooks = mod

    so_path = "/opt/axon/libaxon_pjrt.so"
    lib = ctypes.CDLL(so_path)
    if not hasattr(lib, "axon_start_nrt_profile"):
        return
    lib.axon_start_nrt_profile.argtypes = [
        ctypes.POINTER(ctypes.c_int64),
        ctypes.c_size_t,
    ]
    lib.axon_start_nrt_profile.restype = ctypes.c_int64
    lib.axon_stop_nrt_profile.argtypes = [ctypes.c_char_p]
    lib.axon_stop_nrt_profile.restype = ctypes.c_int64

    @contextlib.contextmanager
    def _hook(output_dir, device_ids):
        import jax

        jax.devices()
        if device_ids:
            ids = (ctypes.c_int64 * len(device_ids))(*device_ids)
            rc = lib.axon_start_nrt_profile(ids, len(device_ids))
        else:
            rc = lib.axon_start_nrt_profile(None, 0)
        if rc != 0:
            raise RuntimeError(f"axon_start_nrt_profile rc={rc}")
        try:
            yield
        finally:
            n = lib.axon_stop_nrt_profile(str(output_dir).encode())
            print(f"profile: {n} file(s) written to {output_dir}")

    mod.set_axon_ntff_profile_hook(_hook)


def _run(in_maps, trace=False):
    if trace:
        _install_axon_profile_shim()
    nc = _get_nc()
    return run_bass_kernel_spmd(nc, in_maps, list(range(N_CORES)), trace=trace)


def kernel(pos_u, pos_w, neg_w, u_emb, w_emb):
    in_maps = _make_in_maps(pos_u, pos_w, neg_w, u_emb, w_emb)
    bkr = _run(in_maps, trace=False)
    total = 0.0
    for r in bkr.results:
        total += float(r["loss_part"].astype(np.float64).sum())
    return np.float32(total)


def kernel_traced(pos_u, pos_w, neg_w, u_emb, w_emb):
    """Like kernel() but returns (loss, BassKernelResults) with HW profile."""
    in_maps = _make_in_maps(pos_u, pos_w, neg_w, u_emb, w_emb)
    bkr = _run(in_maps, trace=True)
    total = 0.0
    for r in bkr.results:
        total += float(r["loss_part"].astype(np.float64).sum())
    return np.float32(total), bkr
